# revision 25
# baseline (speedup 1.0000x reference)
"""Trainium2 Bass kernel for BilinearSeqAttnAction (moe_routing).

Math (per sample b, with a = actions[b]):
    W    = weight * sigmoid(wa[a])                  # [H, H]
    Wy   = y[b] @ W + bias * sigmoid(ba[a])         # [H]
    xWy  = x[b] @ Wy                                # [L]
    out  = softmax(where(mask, -1e30, xWy))         # [L]

Strategy "dpg" (default): data-parallel over samples (core c <-> sample c),
single-pass fp16 matmuls with the sigmoid linearized into a small fp8
correction.  |wa| <= 1/sqrt(H) ~ 0.022, so sigmoid(wa) = 0.5 + wa/4 to
within 2.3e-7 absolute; stage 1 becomes
    Wy = (0.5 y) @ weight + (y/64) @ (weight .* fp8(16*wa)) + bias*sigmoid(ba)
Per-core HBM traffic is 20MB (x 8MB fp16 + weight 8MB fp16 + wa 4MB... 2MB
fp8 + consts) vs 40MB for the bf16 hi/lo baseline, and the kernel is
DMA-bound: ~88us vs 162us baseline at comparable clock.

Strategies "dpf" (two-pass all-fp16) and "dpb" (all-bf16 hi/lo baseline)
are kept for A/B via BASS_KERNEL_STRATEGY.
"""

import os
import sys

sys.path.insert(0, "/opt/trn_rl_repo")

import numpy as np

from concourse import bacc, bass, mybir, tile
from concourse.bass_utils import run_bass_kernel_spmd

F32 = mybir.dt.float32
F16 = mybir.dt.float16
BF16 = mybir.dt.bfloat16
F8 = mybir.dt.float8e4
NP_BF16 = mybir.dt.np(BF16)
NP_F16 = np.float16
NP_F8 = mybir.dt.np(F8)
WA_SCALE = 16.0          # host stages fp8(wa*16); device applies 0.25/16

B, L, H = 8, 2048, 2048
N_ACTIONS = 8
NCORES = 8
P = 128                  # SBUF partitions
FC = 512                 # psum bank chunk (fp32)
NEG_INF = -1e30
MASK_NEG = -30000.0      # fp16-safe stand-in for -1e30 in the mask row

_cache: dict = {}


def _build_dpg():
    """fp16 two-pass data-parallel program, wa in fp8.

    Wy = (0.5 y) @ W + (y/64) @ (W .* wa')   with wa' = fp8(16*wa), so the
    correction pass carries sigmoid'(0)=1/4 exactly:  (y/64)@(W.*16wa) =
    0.25 * y@(W.*wa).  Pass 1 consumes W tiles straight off the DMA queue
    (no elementwise preprocessing on the critical path); pass 2 needs one
    VectorE multiply (fp16 x fp8 -> fp16) per tile.  The correction term is
    ~160x smaller than the main term, so fp8's ~4% rounding on wa costs only
    ~1e-3 relative on the softmax output.  Dummy matmuls keep the PE p-state
    ramped through the DMA-paced stage-1 stream.  Softmax skips the max-shift
    (|logits| < ~60 << 88: exp cannot overflow in fp32).
    """
    nts = H // P
    ntx = H // P
    nlc = L // FC
    nwt = H // P

    nc = bacc.Bacc(
        "TRN2", target_bir_lowering=False, debug=False, num_devices=NCORES
    )

    xt_d = nc.dram_tensor("xt", [ntx, P, L], F16, kind="ExternalInput")
    wt_d = nc.dram_tensor("wt", [nts, P, H], F16, kind="ExternalInput")
    was_d = nc.dram_tensor("was", [nts, P, H], F8, kind="ExternalInput")
    yh_d = nc.dram_tensor("yh", [P, nwt], F16, kind="ExternalInput")  # 0.5*y
    yq_d = nc.dram_tensor("yq", [P, nwt], F16, kind="ExternalInput")  # y/64
    biaspt_d = nc.dram_tensor("biaspt", [P, nwt], F32, kind="ExternalInput")
    bapt_d = nc.dram_tensor("bapt", [P, nwt], F32, kind="ExternalInput")
    mrow_d = nc.dram_tensor("mrow", [1, L], F16, kind="ExternalInput")
    out_d = nc.dram_tensor("out", [1, L], F32, kind="ExternalOutput")

    with tile.TileContext(nc) as tc:
        with (
            tc.tile_pool(name="const", bufs=1) as const_pool,
            tc.tile_pool(name="xp", bufs=min(ntx, 16)) as x_pool,
            tc.tile_pool(name="wp", bufs=10) as w_pool,
            tc.tile_pool(name="wasp", bufs=14) as was_pool,
            tc.tile_pool(name="sigp", bufs=5) as sig_pool,
            tc.tile_pool(name="zp", bufs=5) as z_pool,
            tc.tile_pool(name="pacc", bufs=1, space="PSUM") as pacc_pool,
            tc.tile_pool(name="pd", bufs=1, space="PSUM") as pd_pool,
            tc.tile_pool(name="dram", bufs=1, space="DRAM") as dram_pool,
        ):
            yh = const_pool.tile([P, nwt], F16, tag="yh")
            nc.gpsimd.dma_start(yh[:], yh_d[:])
            yq = const_pool.tile([P, nwt], F16, tag="yq")
            nc.gpsimd.dma_start(yq[:], yq_d[:])
            biaspt = const_pool.tile([P, nwt], F32, tag="biaspt")
            nc.gpsimd.dma_start(biaspt[:], biaspt_d[:])
            bapt = const_pool.tile([P, nwt], F32, tag="bapt")
            nc.gpsimd.dma_start(bapt[:], bapt_d[:])
            mrow = const_pool.tile([1, L], F16, tag="mrow")
            nc.gpsimd.dma_start(mrow[:], mrow_d[:])
            ones1 = const_pool.tile([1, 1], F32, tag="ones1")
            nc.gpsimd.memset(ones1[:], 1.0)
            onesm = const_pool.tile([1, 1], F16, tag="onesm")
            nc.gpsimd.memset(onesm[:], 1.0)
            # dummy operands for PE p-state warmers
            dumr = const_pool.tile([P, FC], F16, tag="dumr")
            nc.gpsimd.memset(dumr[:], 0.0)

            def dummy_mms(n):
                for _ in range(n):
                    pd = pd_pool.tile([1, FC], F32, tag="pd")
                    nc.tensor.matmul(
                        pd[:], dumr[:, 0:1], dumr[:],
                        start=True, stop=True, skip_group_check=True,
                    )

            # ramp the PE before the first W tile lands
            dummy_mms(14)

            # ---- stage 1: Wy = (0.5y)@W + (y/64)@(W .* wa'), two fp16 passes --
            psum_wy = pacc_pool.tile([1, H], F32, tag="pacc")
            for s in range(nts):
                wt = w_pool.tile([P, H], F16, tag="wt")
                wa = was_pool.tile([P, H], F8, tag="was")
                if s % 2 == 0:
                    nc.sync.dma_start(wt[:], wt_d[s])
                    nc.gpsimd.dma_start(wa[:], was_d[s])
                else:
                    nc.gpsimd.dma_start(wt[:], wt_d[s])
                    nc.sync.dma_start(wa[:], was_d[s])
                first = s == 0
                last = s == nts - 1
                for fc in range(nlc):
                    sl = slice(fc * FC, (fc + 1) * FC)
                    nc.tensor.matmul(
                        psum_wy[:, sl], yh[:, s : s + 1], wt[:, sl],
                        start=first, stop=False, skip_group_check=True,
                    )
                # split z = wt .* wa within each tile: ScalarE casts the low
                # half fp8->fp16 (VectorE then multiplies it in 2x 16-bit
                # mode); VectorE mixed-multiplies the raw high half at 1x.
                # Keeps every pipeline stage under the ~2us DMA arrival pace.
                zd = z_pool.tile([P, H], F16, tag="zd")
                CS = 5 * H // 8   # cast split: ScalarE ~1.3us, VectorE ~1.5us
                sig = sig_pool.tile([P, CS], F16, tag="sig")
                nc.scalar.activation(
                    sig[:], wa[:, :CS],
                    mybir.ActivationFunctionType.Copy,
                )
                nc.vector.tensor_mul(zd[:, :CS], wt[:, :CS], sig[:])
                nc.vector.tensor_mul(zd[:, CS:], wt[:, CS:], wa[:, CS:])
                for fc in range(nlc):
                    sl = slice(fc * FC, (fc + 1) * FC)
                    nc.tensor.matmul(
                        psum_wy[:, sl], yq[:, s : s + 1], zd[:, sl],
                        start=False, stop=last, skip_group_check=True,
                    )

            # ---- x stream, after the stage-1 streams on both queues ----
            # the last two tiles stream per-chunk so their matmuls overlap
            # the tiles' own transfer (the only un-overlapped bytes)
            x_tiles = []
            for t in range(ntx):
                xt = x_pool.tile([P, L], F16, tag="xt")
                q = nc.sync if t % 2 == 0 else nc.gpsimd
                if t >= ntx - 2:
                    for lc in range(nlc):
                        sl = slice(lc * FC, (lc + 1) * FC)
                        q.dma_start(xt[:, sl], xt_d[t][:, sl])
                else:
                    q.dma_start(xt[:], xt_d[t])
                x_tiles.append(xt)

            # keep the PE busy (and ramped) while ScalarE copies psum -> SBUF
            dummy_mms(10)

            # bterm = bias * sigmoid(ba): emitted after the cast chain so the
            # Copy table loads first and the chain starts as soon as wa[0] lands
            bterm = const_pool.tile([P, nwt], F32, tag="bterm")
            nc.scalar.activation(
                bterm[:], bapt[:], mybir.ActivationFunctionType.Sigmoid
            )
            nc.vector.tensor_mul(bterm[:], bterm[:], biaspt[:])

            # ---- Wy [1, H] -> lhsT layout [P, nwt] via a DRAM round-trip ----
            # (cheaper than 16 rank-1 PE transposes; rides the scalar-engine
            # HWDGE queue so it does not sit behind the x stream)
            partial = const_pool.tile([1, H], F32, tag="partial")
            nc.scalar.activation(
                partial[:], psum_wy[:], mybir.ActivationFunctionType.Copy
            )
            # pre-warm the Exp table now that all other ScalarE table users ran
            warm = const_pool.tile([1, 1], F32, tag="warm")
            nc.scalar.activation(
                warm[:], ones1[:], mybir.ActivationFunctionType.Exp
            )
            wy_row = dram_pool.tile([1, H], F32, tag="wyrow")
            nc.scalar.dma_start(wy_row[:], partial[:])
            wy_sb = const_pool.tile([P, nwt], F32, tag="wy_sb")
            nc.scalar.dma_start(
                wy_sb[:], wy_row[0].rearrange("(t p) -> p t", p=P)
            )
            wyf = const_pool.tile([P, nwt], F32, tag="wyf")
            nc.vector.tensor_add(wyf[:], wy_sb[:], bterm[:])
            wyP = const_pool.tile([P, nwt], F16, tag="wyP")
            nc.vector.tensor_copy(wyP[:], wyf[:])

            # ---- stage 2: xWy[1, L] += wyP_t^T @ xt_t, fp16 ----
            # additive mask first (x-independent) so the tail ends on the
            # last x-tile matmul
            psum_x = pacc_pool.tile([1, L], F32, tag="pacc")
            for lc in range(nlc):
                sl = slice(lc * FC, (lc + 1) * FC)
                nc.tensor.matmul(
                    psum_x[:, sl], onesm[:], mrow[0:1, sl],
                    start=True, stop=False, skip_group_check=True,
                )
            for t in range(ntx):
                for lc in range(nlc):
                    sl = slice(lc * FC, (lc + 1) * FC)
                    nc.tensor.matmul(
                        psum_x[:, sl], wyP[:, t : t + 1], x_tiles[t][:, sl],
                        start=False, stop=(t == ntx - 1), skip_group_check=True,
                    )

            # ---- softmax without max-shift: logits bounded well below 88 ----
            # per-chunk Exp pipelines behind each PSUM bank's final matmul
            # (the last x tile streams per-chunk, so banks stop ~400ns apart)
            exps = const_pool.tile([1, L], F32, tag="exps")
            sums = const_pool.tile([1, nlc], F32, tag="sums")
            for lc in range(nlc):
                sl = slice(lc * FC, (lc + 1) * FC)
                nc.scalar.activation(
                    exps[:, sl], psum_x[0:1, sl],
                    mybir.ActivationFunctionType.Exp,
                    accum_out=sums[:, lc : lc + 1],
                )
            sume = const_pool.tile([1, 1], F32, tag="sume")
            nc.vector.tensor_reduce(
                sume[:], sums[:], axis=mybir.AxisListType.X,
                op=mybir.AluOpType.add,
            )
            rinv = const_pool.tile([1, 1], F32, tag="rinv")
            nc.vector.reciprocal(rinv[:], sume[:])
            # normalize + store in halves: first DMA overlaps second multiply
            outrow = const_pool.tile([1, L], F32, tag="outrow")
            HL = L // 2
            nc.vector.tensor_scalar_mul(
                outrow[:, :HL], exps[:, :HL], rinv[:, 0:1]
            )
            nc.scalar.dma_start(out_d[:, :HL], outrow[:, :HL])
            nc.vector.tensor_scalar_mul(
                outrow[:, HL:], exps[:, HL:], rinv[:, 0:1]
            )
            nc.scalar.dma_start(out_d[:, HL:], outrow[:, HL:])

    nc.compile()
    return nc


def _build_dpf():
    """fp16 single-pass data-parallel program (one sample per core)."""
    nts = H // P   # weight h2 tiles
    ntx = H // P   # x h tiles
    nlc = L // FC
    nwt = H // P

    nc = bacc.Bacc(
        "TRN2", target_bir_lowering=False, debug=False, num_devices=NCORES
    )

    xt_d = nc.dram_tensor("xt", [ntx, P, L], F16, kind="ExternalInput")
    wt_d = nc.dram_tensor("wt", [nts, P, H], F16, kind="ExternalInput")
    was_d = nc.dram_tensor("was", [nts, P, H], F16, kind="ExternalInput")
    yh_d = nc.dram_tensor("yh", [P, nwt], F16, kind="ExternalInput")   # 0.5*y
    yq_d = nc.dram_tensor("yq", [P, nwt], F16, kind="ExternalInput")   # 0.25*y
    biaspt_d = nc.dram_tensor("biaspt", [P, nwt], F32, kind="ExternalInput")
    bapt_d = nc.dram_tensor("bapt", [P, nwt], F32, kind="ExternalInput")
    mrow_d = nc.dram_tensor("mrow", [1, L], F16, kind="ExternalInput")
    out_d = nc.dram_tensor("out", [1, L], F32, kind="ExternalOutput")

    with tile.TileContext(nc) as tc:
        with (
            tc.tile_pool(name="const", bufs=1) as const_pool,
            tc.tile_pool(name="xp", bufs=min(ntx, 16)) as x_pool,
            tc.tile_pool(name="wp", bufs=3) as w_pool,
            tc.tile_pool(name="wasp", bufs=3) as was_pool,
            tc.tile_pool(name="zp", bufs=3) as z_pool,
            tc.tile_pool(name="pacc", bufs=1, space="PSUM") as pacc_pool,
            tc.tile_pool(name="pt", bufs=1, space="PSUM") as pt_pool,
        ):
            # small constants on the gpsimd queue, ahead of the was stream
            yh = const_pool.tile([P, nwt], F16, tag="yh")
            nc.gpsimd.dma_start(yh[:], yh_d[:])
            yq = const_pool.tile([P, nwt], F16, tag="yq")
            nc.gpsimd.dma_start(yq[:], yq_d[:])
            biaspt = const_pool.tile([P, nwt], F32, tag="biaspt")
            nc.gpsimd.dma_start(biaspt[:], biaspt_d[:])
            bapt = const_pool.tile([P, nwt], F32, tag="bapt")
            nc.gpsimd.dma_start(bapt[:], bapt_d[:])
            mrow = const_pool.tile([1, L], F16, tag="mrow")
            nc.gpsimd.dma_start(mrow[:], mrow_d[:])
            ones1 = const_pool.tile([1, 1], F32, tag="ones1")
            nc.gpsimd.memset(ones1[:], 1.0)
            onesm = const_pool.tile([1, 1], F16, tag="onesm")
            nc.gpsimd.memset(onesm[:], 1.0)

            # ---- stage 1: Wy = (0.5y)@w + (0.25y)@(w .* wa) ----
            psum_wy = pacc_pool.tile([1, H], F32, tag="pacc")
            for s in range(nts):
                wt = w_pool.tile([P, H], F16, tag="wt")
                nc.sync.dma_start(wt[:], wt_d[s])
                wa = was_pool.tile([P, H], F16, tag="was")
                nc.gpsimd.dma_start(wa[:], was_d[s])
                zd = z_pool.tile([P, H], F16, tag="zd")
                nc.vector.tensor_mul(zd[:], wa[:], wt[:])
                first = s == 0
                last = s == nts - 1
                for fc in range(nlc):
                    sl = slice(fc * FC, (fc + 1) * FC)
                    nc.tensor.matmul(
                        psum_wy[:, sl], yh[:, s : s + 1], wt[:, sl],
                        start=first, stop=False, skip_group_check=True,
                    )
                    nc.tensor.matmul(
                        psum_wy[:, sl], yq[:, s : s + 1], zd[:, sl],
                        start=False, stop=last, skip_group_check=True,
                    )

            # ---- x stream, queued after the stage-1 streams on both queues ----
            x_tiles = []
            for t in range(ntx):
                xt = x_pool.tile([P, L], F16, tag="xt")
                if t % 2 == 0:
                    nc.sync.dma_start(xt[:], xt_d[t])
                else:
                    nc.gpsimd.dma_start(xt[:], xt_d[t])
                x_tiles.append(xt)

            # bterm = bias * sigmoid(ba)  [P, nwt]
            bterm = const_pool.tile([P, nwt], F32, tag="bterm")
            nc.scalar.activation(
                bterm[:], bapt[:], mybir.ActivationFunctionType.Sigmoid
            )
            nc.vector.tensor_mul(bterm[:], bterm[:], biaspt[:])

            # ---- Wy [1, H] -> lhsT layout [P, nwt] via 16 rank-1 transposes ----
            partial = const_pool.tile([1, H], F32, tag="partial")
            nc.scalar.activation(
                partial[:], psum_wy[:], mybir.ActivationFunctionType.Copy
            )
            psum_t = pt_pool.tile([P, nwt], F32, tag="pt")
            for t in range(nwt):
                nc.tensor.matmul(
                    psum_t[:, t : t + 1],
                    partial[0:1, t * P : (t + 1) * P],
                    ones1[:],
                    start=True, stop=True, skip_group_check=True,
                )
            wyf = const_pool.tile([P, nwt], F32, tag="wyf")
            nc.vector.tensor_add(wyf[:], psum_t[:], bterm[:])
            wyP = const_pool.tile([P, nwt], F16, tag="wyP")
            nc.vector.tensor_copy(wyP[:], wyf[:])

            # ---- stage 2: xWy[1, L] += wyP_t^T @ xt_t, fp16 ----
            psum_x = pacc_pool.tile([1, L], F32, tag="pacc")
            for t in range(ntx):
                for lc in range(nlc):
                    sl = slice(lc * FC, (lc + 1) * FC)
                    nc.tensor.matmul(
                        psum_x[:, sl], wyP[:, t : t + 1], x_tiles[t][:, sl],
                        start=(t == 0), stop=False, skip_group_check=True,
                    )
            # additive mask folded into PSUM as a rank-1 update (1 x mrow)
            for lc in range(nlc):
                sl = slice(lc * FC, (lc + 1) * FC)
                nc.tensor.matmul(
                    psum_x[:, sl], onesm[:], mrow[0:1, sl],
                    start=False, stop=(lc == nlc - 1), skip_group_check=True,
                )

            # ---- softmax on [1, L] straight out of PSUM ----
            negmx = const_pool.tile([1, 1], F32, tag="negmx")
            nc.vector.tensor_reduce(
                negmx[:], psum_x[0:1, :], axis=mybir.AxisListType.X,
                op=mybir.AluOpType.max, negate=True,
            )
            exps = const_pool.tile([1, L], F32, tag="exps")
            sume = const_pool.tile([1, 1], F32, tag="sume")
            nc.scalar.activation(
                exps[:], psum_x[0:1, :], mybir.ActivationFunctionType.Exp,
                bias=negmx[:, 0:1], scale=1.0, accum_out=sume[:],
            )
            rinv = const_pool.tile([1, 1], F32, tag="rinv")
            nc.vector.reciprocal(rinv[:], sume[:])
            outrow = const_pool.tile([1, L], F32, tag="outrow")
            nc.vector.tensor_scalar_mul(outrow[:], exps[:], rinv[:, 0:1])
            nc.gpsimd.dma_start(out_d[:], outrow[:])

    nc.compile()
    return nc


def _build_dpb():
    """All-bf16-matmul data-parallel program (one sample per core).

    sigmoid(v) = 0.5 + 0.5*tanh(v/2), so with d := w_hi * (0.5*tanh(wa/2)):
        Wy = (0.5*y) @ (w_hi + w_lo) + y @ d + bias*sigmoid(ba)
    where w = w_hi + w_lo is an exact bf16 hi/lo split done on the host.
    Stage 2 likewise splits x (host) and Wy (device) into bf16 hi/lo and
    accumulates three bf16 passes (hi*hi + hi*lo + lo*hi) in fp32 PSUM.
    """
    nts = H // P   # weight h tiles
    ntx = H // P   # x h tiles
    nlc = L // FC
    nwt = H // P

    nc = bacc.Bacc(
        "TRN2", target_bir_lowering=False, debug=False, num_devices=NCORES
    )

    xh_d = nc.dram_tensor("xh", [ntx, P, L], BF16, kind="ExternalInput")
    xl_d = nc.dram_tensor("xl", [ntx, P, L], BF16, kind="ExternalInput")
    wh_d = nc.dram_tensor("wh", [nts, P, H], BF16, kind="ExternalInput")
    wl_d = nc.dram_tensor("wl", [nts, P, H], BF16, kind="ExternalInput")
    was_d = nc.dram_tensor("was", [nts, P, H], BF16, kind="ExternalInput")
    # y12[:, s, 0] = bf16(y/2), y12[:, s, 1] = bf16(y/2 - hi) -- packed so one
    # P_out=2 matmul runs both hi and lo passes against the shared rhs wh
    y12_d = nc.dram_tensor("y12", [P, nwt, 2], BF16, kind="ExternalInput")
    y1h_d = nc.dram_tensor("y1h", [P, nwt], BF16, kind="ExternalInput")
    y2_d = nc.dram_tensor("y2", [P, nwt], BF16, kind="ExternalInput")
    biaspt_d = nc.dram_tensor("biaspt", [P, nwt], F32, kind="ExternalInput")
    bapt_d = nc.dram_tensor("bapt", [P, nwt], F32, kind="ExternalInput")
    mrow_d = nc.dram_tensor("mrow", [1, L], BF16, kind="ExternalInput")
    out_d = nc.dram_tensor("out", [1, L], F32, kind="ExternalOutput")

    with tile.TileContext(nc) as tc:
        with (
            tc.tile_pool(name="const", bufs=1) as const_pool,
            tc.tile_pool(name="xhp", bufs=min(ntx, 15)) as xh_pool,
            tc.tile_pool(name="xlp", bufs=min(ntx, 15)) as xl_pool,
            tc.tile_pool(name="whp", bufs=3) as wh_pool,
            tc.tile_pool(name="wlp", bufs=3) as wl_pool,
            tc.tile_pool(name="wasp", bufs=3) as was_pool,
            tc.tile_pool(name="tnhp", bufs=3) as tnh_pool,
            tc.tile_pool(name="zdp", bufs=3) as zd_pool,
            tc.tile_pool(name="pacc", bufs=1, space="PSUM") as psum_acc_pool,
            tc.tile_pool(name="pwyt", bufs=1, space="PSUM") as psum_wyt_pool,
        ):
            psum_wy_pool = psum_x_pool = psum_acc_pool
            # stage-1-critical small constants only; the rest load after x
            y12 = const_pool.tile([P, nwt, 2], BF16, tag="y12")
            nc.gpsimd.dma_start(y12[:], y12_d[:])
            y1h = const_pool.tile([P, nwt], BF16, tag="y1h")
            nc.gpsimd.dma_start(y1h[:], y1h_d[:])
            y2 = const_pool.tile([P, nwt], BF16, tag="y2")
            nc.gpsimd.dma_start(y2[:], y2_d[:])

            # ---- stage 1 ----
            psum_wy = psum_wy_pool.tile([2, H], F32, tag="pacc")
            for s in range(nts):
                wh = wh_pool.tile([P, H], BF16, tag="wh")
                nc.sync.dma_start(wh[:], wh_d[s])
                wat = was_pool.tile([P, H], BF16, tag="was")
                nc.gpsimd.dma_start(wat[:], was_d[s])
                wl = wl_pool.tile([P, H], BF16, tag="wl")
                nc.sync.dma_start(wl[:, : H // 2], wl_d[s][:, : H // 2])
                nc.gpsimd.dma_start(wl[:, H // 2 :], wl_d[s][:, H // 2 :])
                tnh = tnh_pool.tile([P, H], BF16, tag="tnh")
                nc.scalar.activation(
                    tnh[:], wat[:], mybir.ActivationFunctionType.Tanh, scale=0.5
                )
                zd = zd_pool.tile([P, H], BF16, tag="zd")
                nc.vector.scalar_tensor_tensor(
                    zd[:], wh[:], 0.5, tnh[:],
                    mybir.AluOpType.mult, mybir.AluOpType.mult,
                )
                first = s == 0
                last = s == nts - 1
                for fc in range(H // FC):
                    sl = slice(fc * FC, (fc + 1) * FC)
                    nc.tensor.matmul(
                        psum_wy[:, sl], y12[:, s, :], wh[:, sl],
                        start=first, stop=False, skip_group_check=True,
                    )
                    nc.tensor.matmul(
                        psum_wy[0:1, sl], y1h[:, s : s + 1], wl[:, sl],
                        start=False, stop=False, skip_group_check=True,
                    )
                    nc.tensor.matmul(
                        psum_wy[0:1, sl], y2[:, s : s + 1], zd[:, sl],
                        start=False, stop=last, skip_group_check=True,
                    )

            biaspt = const_pool.tile([P, nwt], F32, tag="biaspt")
            nc.sync.dma_start(biaspt[:], biaspt_d[:])
            bapt = const_pool.tile([P, nwt], F32, tag="bapt")
            nc.sync.dma_start(bapt[:], bapt_d[:])
            mrow = const_pool.tile([1, L], BF16, tag="mrow")
            nc.sync.dma_start(mrow[:], mrow_d[:])
            ones2 = const_pool.tile([2, 1], F32, tag="ones2")
            nc.gpsimd.memset(ones2[:], 1.0)
            ones_bf = const_pool.tile([1, 1], BF16, tag="ones_bf")
            nc.gpsimd.memset(ones_bf[:], 1.0)

            # ---- x stream, queued after the stage-1 streams on both queues ----
            xh_tiles, xl_tiles = [], []
            for t in range(ntx):
                xh = xh_pool.tile([P, L], BF16, tag="xh")
                nc.sync.dma_start(xh[:], xh_d[t])
                xh_tiles.append(xh)
                xl = xl_pool.tile([P, L], BF16, tag="xl")
                if t == ntx - 1:
                    nc.sync.dma_start(xl[:], xl_d[t])
                else:
                    nc.gpsimd.dma_start(xl[:], xl_d[t])
                xl_tiles.append(xl)

            # bterm = bias * (0.5 + 0.5*tanh(ba/2))   [P, nwt]
            bterm = const_pool.tile([P, nwt], F32, tag="bterm")
            nc.scalar.activation(
                bterm[:], bapt[:], mybir.ActivationFunctionType.Tanh, scale=0.5
            )
            nc.vector.tensor_scalar(
                bterm[:], bterm[:], 0.5, 0.5,
                mybir.AluOpType.mult, mybir.AluOpType.add,
            )
            nc.vector.tensor_mul(bterm[:], bterm[:], biaspt[:])

            # ---- Wy -> [P, nwt] lhsT layout, + bias term, hi/lo split ----
            partial2 = const_pool.tile([2, H], F32, tag="rowbuf")
            nc.scalar.activation(
                partial2[:], psum_wy[:], mybir.ActivationFunctionType.Copy
            )
            psum_t = psum_wyt_pool.tile([P, nwt], F32, tag="pwyt")
            for t in range(nwt):
                nc.tensor.matmul(
                    psum_t[:, t : t + 1],
                    partial2[0:2, t * P : (t + 1) * P],
                    ones2[:],
                    start=True, stop=True, skip_group_check=True,
                )
            wyf = const_pool.tile([P, nwt], F32, tag="wyf")
            nc.vector.tensor_add(wyf[:], psum_t[:], bterm[:])
            wyP = const_pool.tile([P, nwt, 2], BF16, tag="wyP")
            nc.vector.tensor_copy(wyP[:, :, 0], wyf[:])
            wy_hi32 = const_pool.tile([P, nwt], F32, tag="wy_hi32")
            nc.vector.tensor_copy(wy_hi32[:], wyP[:, :, 0])
            wy_lo32 = const_pool.tile([P, nwt], F32, tag="wy_lo32")
            nc.vector.tensor_sub(wy_lo32[:], wyf[:], wy_hi32[:])
            nc.vector.tensor_copy(wyP[:, :, 1], wy_lo32[:])

            # ---- stage 2: two bf16 passes per (t, chunk), rows in psum ----
            psum_x = psum_x_pool.tile([2, L], F32, tag="pacc")
            for t in range(ntx):
                first = t == 0
                for lc in range(nlc):
                    sl = slice(lc * FC, (lc + 1) * FC)
                    nc.tensor.matmul(
                        psum_x[:, sl], wyP[:, t, :], xh_tiles[t][:, sl],
                        start=first, stop=False, skip_group_check=True,
                    )
                    nc.tensor.matmul(
                        psum_x[0:1, sl], wyP[:, t, 0:1], xl_tiles[t][:, sl],
                        start=False, stop=False, skip_group_check=True,
                    )
            for lc in range(nlc):
                sl = slice(lc * FC, (lc + 1) * FC)
                nc.tensor.matmul(
                    psum_x[0:1, sl], ones_bf[:], mrow[0:1, sl],
                    start=False, stop=(lc == nlc - 1), skip_group_check=True,
                )

            # ---- combine the two psum rows, then softmax on [1, L] ----
            sb2 = const_pool.tile([2, L], F32, tag="rowbuf")
            nc.scalar.activation(
                sb2[:], psum_x[:], mybir.ActivationFunctionType.Copy
            )
            nc.gpsimd.dma_start(
                sb2[0:1, :], sb2[1:2, :], accum_op=mybir.AluOpType.add
            )
            negmx = const_pool.tile([1, 1], F32, tag="negmx")
            nc.vector.tensor_reduce(
                negmx[:], psum_x[0:1, :], axis=mybir.AxisListType.X,
                op=mybir.AluOpType.max, negate=True,
            )
            exps = const_pool.tile([1, L], F32, tag="exps")
            sume = const_pool.tile([1, 1], F32, tag="sume")
            nc.scalar.activation(
                exps[:], sb2[0:1, :], mybir.ActivationFunctionType.Exp,
                bias=negmx[:, 0:1], scale=1.0, accum_out=sume[:],
            )
            rinv = const_pool.tile([1, 1], F32, tag="rinv")
            nc.vector.reciprocal(rinv[:], sume[:])
            outrow = const_pool.tile([1, L], F32, tag="rowbuf")
            nc.vector.tensor_scalar_mul(outrow[:], exps[:], rinv[:, 0:1])
            nc.gpsimd.dma_start(out_d[:], outrow[:])

    nc.compile()
    return nc


def kernel(x, y, x_mask, actions, weight, bias, wa, ba):
    x = np.asarray(x, dtype=np.float32)
    y = np.asarray(y, dtype=np.float32)
    x_mask = np.asarray(x_mask)
    actions = np.asarray(actions).astype(np.int64)
    weight = np.asarray(weight, dtype=np.float32)
    bias = np.asarray(bias, dtype=np.float32)
    wa = np.asarray(wa, dtype=np.float32)
    ba = np.asarray(ba, dtype=np.float32)

    strategy = os.environ.get("BASS_KERNEL_STRATEGY", "dpg")

    nts = H // P
    nwt = H // P
    ntx = H // P

    if strategy not in _cache:
        _cache[strategy] = {
            "dpg": _build_dpg,
            "dpf": _build_dpf,
            "dpb": _build_dpb,
        }[strategy]()
    nc = _cache[strategy]

    if strategy == "dpg":
        wt = weight.astype(NP_F16).reshape(nts, P, H)
        wa8 = (wa * WA_SCALE).astype(NP_F8)
        in_maps = []
        for c in range(NCORES):
            a = int(actions[c])
            m = {
                "xt": np.ascontiguousarray(x[c].T).astype(NP_F16).reshape(
                    ntx, P, L
                ),
                "wt": wt,
                "was": wa8[a].reshape(nts, P, H),
                "yh": np.ascontiguousarray(
                    (0.5 * y[c]).astype(NP_F16).reshape(nwt, P).T
                ),
                "yq": np.ascontiguousarray(
                    (y[c] / (4.0 * WA_SCALE)).astype(NP_F16).reshape(nwt, P).T
                ),
                "biaspt": np.ascontiguousarray(bias.reshape(nwt, P).T),
                "bapt": np.ascontiguousarray(ba[a].reshape(nwt, P).T),
                "mrow": np.where(
                    x_mask[c], np.float32(MASK_NEG), np.float32(0.0)
                )[None, :].astype(NP_F16),
            }
            in_maps.append(m)
        return _run(nc, in_maps)

    if strategy == "dpf":
        wt = weight.astype(NP_F16).reshape(nts, P, H)
        wa16 = wa.astype(NP_F16)
        in_maps = []
        for c in range(NCORES):
            a = int(actions[c])
            m = {
                "xt": np.ascontiguousarray(x[c].T).astype(NP_F16).reshape(
                    ntx, P, L
                ),
                "wt": wt,
                "was": wa16[a].reshape(nts, P, H),
                "yh": np.ascontiguousarray(
                    (0.5 * y[c]).astype(NP_F16).reshape(nwt, P).T
                ),
                "yq": np.ascontiguousarray(
                    (0.25 * y[c]).astype(NP_F16).reshape(nwt, P).T
                ),
                "biaspt": np.ascontiguousarray(bias.reshape(nwt, P).T),
                "bapt": np.ascontiguousarray(ba[a].reshape(nwt, P).T),
                "mrow": np.where(
                    x_mask[c], np.float32(MASK_NEG), np.float32(0.0)
                )[None, :].astype(NP_F16),
            }
            in_maps.append(m)
        return _run(nc, in_maps)

    # ---- dpb fallback ----
    wh32 = weight.astype(NP_BF16).astype(np.float32)
    wh = wh32.astype(NP_BF16).reshape(nts, P, H)
    wl = (weight - wh32).astype(NP_BF16).reshape(nts, P, H)
    in_maps = []
    for c in range(NCORES):
        a = int(actions[c])
        xt = np.ascontiguousarray(x[c].T)
        xh32 = xt.astype(NP_BF16).astype(np.float32)
        yh = 0.5 * y[c]
        yh32 = yh.astype(NP_BF16).astype(np.float32)
        m = {
            "xh": xh32.astype(NP_BF16).reshape(ntx, P, L),
            "xl": (xt - xh32).astype(NP_BF16).reshape(ntx, P, L),
            "wh": wh,
            "wl": wl,
            "was": wa[a].astype(NP_BF16).reshape(nts, P, H),
            "y12": np.ascontiguousarray(
                np.stack(
                    [
                        yh32.astype(NP_BF16).reshape(nwt, P).T,
                        (yh - yh32).astype(NP_BF16).reshape(nwt, P).T,
                    ],
                    axis=-1,
                )
            ),
            "y1h": np.ascontiguousarray(yh32.astype(NP_BF16).reshape(nwt, P).T),
            "y2": np.ascontiguousarray(y[c].astype(NP_BF16).reshape(nwt, P).T),
            "biaspt": np.ascontiguousarray(bias.reshape(nwt, P).T),
            "bapt": np.ascontiguousarray(ba[a].reshape(nwt, P).T),
            "mrow": np.where(x_mask[c], np.float32(NEG_INF), np.float32(0.0))[
                None, :
            ].astype(NP_BF16),
        }
        in_maps.append(m)
    return _run(nc, in_maps)


def _run(nc, in_maps):
    trace = os.environ.get("BASS_KERNEL_TRACE", "0") == "1"
    kwargs = {}
    if trace:
        kwargs["trace"] = True
        tc_env = os.environ.get("BASS_KERNEL_TRACE_CORES", "0")
        kwargs["trace_cores"] = [int(t) for t in tc_env.split(",")]
    res = run_bass_kernel_spmd(nc, in_maps, core_ids=list(range(NCORES)), **kwargs)
    global last_result
    last_result = res
    out = np.stack([res.results[c]["out"][0] for c in range(NCORES)], axis=0)
    return out.astype(np.float32)


last_result = None
